# revision 15
# baseline (speedup 1.0000x reference)
"""DetectionLoss Bass kernel for Trainium2, data-parallel over 8 NeuronCores.

v2 strategy (per core, 8 images as 4 image-pairs):
  - layout: [128 partitions = 2 images x 64 targets, n(preds) free]
  - PE broadcasts pred streams (x2,x1,y2,y1,(3/13)wh as bf16 hi+lo, K=20)
    into PSUM; ScalarE relu-evacuates with per-partition target biases.
  - overlap_x = min(relu(x2-x1g), wg) - relu(x1-x1g)  (DVE STT)
  - ratio r = inter / ((3/13)(a1+a2))  via gpsimd IEEE divide
    (iou > 0.3  <=>  r > 1; argmax_m iou == argmax_m r)
  - argmax over targets WITHOUT gpsimd partition reduce or shift DMAs:
    DVE StreamTranspose (32x32 blocks) -> segmented free-axis max ->
    cross-offset tiny max -> equality mask in transposed space ->
    bf16 StreamTranspose back.
  - matched = (r_best > 1) rides the PE gather matmul as 2 extra
    indicator weight columns, so coords + matched flags come back in one
    [18, n] gather output (coords as bf16 hi+lo rows, exact).
  - focal BCE + CIoU finalization batched over all 8 images.
Host combines the 8x8 per-image (focal_sum, masked_ciou_sum, n_pos).
"""

import numpy as np

import concourse.bass as bass
import concourse.bass_isa as bass_isa
from concourse.bacc import Bacc
import concourse.mybir as mybir
from concourse.tile import TileContext

ALU = mybir.AluOpType
ACT = mybir.ActivationFunctionType
F32 = mybir.dt.float32
BF16 = mybir.dt.bfloat16

# problem constants (hardcoded per harness contract)
B_FULL = 64
N = 8400
M = 64
NCORES = 8
BC = B_FULL // NCORES          # images per core
P = 128
C = 66                          # free cols per partition in n-part layout
NPAD = P * C                    # 8448
NC = 1024                       # n-chunk
CHUNKS = [(k * NC, min(NC, NPAD - k * NC)) for k in range((NPAD + NC - 1) // NC)]
SC13 = 3.0 / 13.0               # iou>0.3  <=>  inter > (3/13)(a1+a2)
EPS = 1e-7

PAD_ROW = np.array([-100.0, -100.0, 1.0, 1.0, -30.0], np.float32)


def pad_preds(preds):
    """Host-side: pad [b, N, 5] -> [b, NPAD, 5] with far-box/low-logit rows."""
    out = np.empty((preds.shape[0], NPAD, 5), np.float32)
    out[:, :N] = preds
    out[:, N:] = PAD_ROW
    return out


def host_consts():
    """Host-built constants: selector matmul weights + per-partition scalars."""
    import ml_dtypes
    # K=20 bf16 selector: rows 0..9 hi streams, 10..19 lo streams; stream s
    # picks rows {2s (img A), 2s+1 (img B)} from both halves.
    sels = np.zeros((20, 5 * P), np.float32)
    for s in range(5):
        for base in (0, 10):
            sels[base + 2 * s, s * P : s * P + 64] = 1.0
            sels[base + 2 * s + 1, s * P + 64 : (s + 1) * P] = 1.0
    sels = sels.astype(ml_dtypes.bfloat16)
    onesneg = np.zeros((P, 2), np.float32)
    onesneg[:, 0] = 1.0
    onesneg[:, 1] = -1.0
    # bf16 matched-indicator columns for the gather weights: col 0 = 1 on
    # img-A partitions (0:64), col 1 = 1 on img-B partitions (64:128)
    onesab = np.zeros((P, 2), np.float32)
    onesab[0:64, 0] = 1.0
    onesab[64:P, 1] = 1.0
    onesab = onesab.astype(ml_dtypes.bfloat16)
    return sels, onesneg, onesab


def build_nc(bc=BC, trn_type=None):
    """Build the per-core Bass program. bc = images per core (even)."""
    pairs = bc // 2
    nc = Bacc() if trn_type is None else Bacc(trn_type=trn_type)
    preds_d = nc.declare_dram_parameter("preds", [bc, NPAD, 5], F32, isOutput=False)
    tgts_d = nc.declare_dram_parameter("targets", [bc, M, 4], F32, isOutput=False)
    sels_d = nc.declare_dram_parameter("sels", [20, 5 * P], BF16, isOutput=False)
    ones_d = nc.declare_dram_parameter("onesneg", [P, 2], F32, isOutput=False)
    onesab_d = nc.declare_dram_parameter("onesab", [P, 2], BF16, isOutput=False)
    out_d = nc.declare_dram_parameter("out", [1, 3 * bc], F32, isOutput=True)

    with TileContext(nc) as tc:
        with (
            tc.tile_pool(name="const", bufs=1) as cpool,
            tc.tile_pool(name="persist", bufs=1) as ppool,
        ):
            SELS = cpool.tile([20, 5 * P], BF16, name="SELS")
            nc.sync.dma_start(out=SELS[:, :], in_=sels_d[:, :])
            ON = cpool.tile([P, 2], F32, name="ON")
            nc.sync.dma_start(out=ON[:, :], in_=ones_d[:, :])
            ONES = ON[:, 0:1]
            NEG1 = ON[:, 1:2]
            OAB = cpool.tile([P, 2], BF16, name="OAB")
            nc.sync.dma_start(out=OAB[:, :], in_=onesab_d[:, :])

            # ---- persistent (all images) ----
            PRED = ppool.tile([P, bc * C * 5], F32, name="PRED")
            # coordinate streams, stream order (X2, X1, Y2, Y1, A1S)
            CRD = ppool.tile([P, bc * 5 * C], F32, name="CRD")
            MT9 = ppool.tile([P, bc * 9 * C], BF16, name="MT9")  # hi+lo coords + flag
            SC = ppool.tile([P, 3 * bc], F32, name="SC")         # accumulators

            crd = CRD.rearrange("p (b s c) -> p b s c", b=bc, s=5)

            with (
                tc.tile_pool(name="stage", bufs=1) as spool,
                tc.tile_pool(name="prep", bufs=2) as qpool,
                tc.tile_pool(name="work", bufs=2) as wpool,
                tc.tile_pool(name="psum", bufs=1, space="PSUM") as pspool,
            ):
                STGB = spool.tile([20, NPAD], BF16, name="STGB", bufs=2)
                STGF = spool.tile([18, NPAD], BF16, name="STGF", bufs=2)

                for pr in range(pairs):
                    bA, bB = 2 * pr, 2 * pr + 1
                    # ================= prep (n-part layout) =================
                    for bslot in (bA, bB):
                        pv = PRED.rearrange("p (b c f) -> p b c f", b=bc, f=5)[:, bslot]
                        src = preds_d[bslot].rearrange("(p c) f -> p c f", c=C)
                        nc.sync.dma_start(out=pv[:, :], in_=src)
                    pv = PRED.rearrange("p (b c f) -> p b c f", b=bc, f=5)

                    for bslot in (bA, bB):
                        cx = pv[:, bslot, :, 0]
                        cy = pv[:, bslot, :, 1]
                        w = pv[:, bslot, :, 2]
                        h = pv[:, bslot, :, 3]
                        WH = qpool.tile([P, C], F32, name="WH", tag="wh", bufs=4)
                        HH = qpool.tile([P, C], F32, name="HH", tag="hh", bufs=4)
                        nc.vector.tensor_scalar(WH[:, :], w, 0.5, None, ALU.mult)
                        nc.vector.tensor_scalar(HH[:, :], h, 0.5, None, ALU.mult)
                        nc.vector.tensor_tensor(crd[:, bslot, 1], cx, WH[:, :], ALU.subtract)
                        nc.vector.tensor_tensor(crd[:, bslot, 0], cx, WH[:, :], ALU.add)
                        nc.vector.tensor_tensor(crd[:, bslot, 3], cy, HH[:, :], ALU.subtract)
                        nc.vector.tensor_tensor(crd[:, bslot, 2], cy, HH[:, :], ALU.add)
                        nc.vector.scalar_tensor_tensor(
                            crd[:, bslot, 4], w, SC13, h, ALU.mult, ALU.mult
                        )

                    # bf16 hi/lo split of the pair's streams, batched scatter
                    CRDH = qpool.tile([P, 2 * 5 * C], BF16, name="CRDH", tag="crdh",
                                      bufs=2)
                    CRDL = qpool.tile([P, 2 * 5 * C], BF16, name="CRDL", tag="crdl",
                                      bufs=2)
                    pair_crd = CRD[:, bA * 5 * C : (bB + 1) * 5 * C]
                    nc.vector.tensor_copy(CRDH[:, :], pair_crd)
                    nc.vector.tensor_tensor(CRDL[:, :], pair_crd, CRDH[:, :],
                                            ALU.subtract)
                    # dst rows r=2s+j; src laid out [p, (j s c)] -> iterate (s j) p c
                    for r0, TSRC in ((0, CRDH), (10, CRDL)):
                        sv = TSRC.rearrange("p (j s c) -> p j s c", j=2, s=5)
                        for s in range(5):
                            for j in range(2):
                                dst = STGB[r0 + 2 * s + j : r0 + 2 * s + j + 1, :]
                                nc.sync.dma_start(
                                    out=dst.rearrange("o (p c) -> o p c", c=C),
                                    in_=sv[:, j, s],
                                )

                    # ---- targets: per-partition scalars (A on 0:64, B on 64:128)
                    TGT = qpool.tile([P, 4], F32, name="TGT", tag="tgt", bufs=3)
                    nc.sync.dma_start(out=TGT[0:64, :], in_=tgts_d[bA])
                    nc.sync.dma_start(out=TGT[64:P, :], in_=tgts_d[bB])
                    TWH = qpool.tile([P, 1], F32, name="TWH", tag="twh")
                    THH = qpool.tile([P, 1], F32, name="THH", tag="thh")
                    TX1 = qpool.tile([P, 1], F32, name="TX1", tag="tx1")
                    TY1 = qpool.tile([P, 1], F32, name="TY1", tag="ty1")
                    TX2 = qpool.tile([P, 1], F32, name="TX2", tag="tx2")
                    TY2 = qpool.tile([P, 1], F32, name="TY2", tag="ty2")
                    NX1 = qpool.tile([P, 1], F32, name="NX1", tag="nx1")
                    NY1 = qpool.tile([P, 1], F32, name="NY1", tag="ny1")
                    A2S = qpool.tile([P, 1], F32, name="A2S", tag="a2s")
                    wg = TGT[:, 2:3]
                    hg = TGT[:, 3:4]
                    nc.vector.tensor_scalar(TWH[:, :], wg, 0.5, None, ALU.mult)
                    nc.vector.tensor_scalar(THH[:, :], hg, 0.5, None, ALU.mult)
                    nc.vector.tensor_tensor(TX1[:, :], TGT[:, 0:1], TWH[:, :], ALU.subtract)
                    nc.vector.tensor_tensor(TX2[:, :], TGT[:, 0:1], TWH[:, :], ALU.add)
                    nc.vector.tensor_tensor(TY1[:, :], TGT[:, 1:2], THH[:, :], ALU.subtract)
                    nc.vector.tensor_tensor(TY2[:, :], TGT[:, 1:2], THH[:, :], ALU.add)
                    nc.vector.tensor_scalar(NX1[:, :], TX1[:, :], -1.0, None, ALU.mult)
                    nc.vector.tensor_scalar(NY1[:, :], TY1[:, :], -1.0, None, ALU.mult)
                    nc.vector.scalar_tensor_tensor(
                        A2S[:, :], wg, SC13, hg, ALU.mult, ALU.mult
                    )
                    # gather weights [P, 18] bf16, 9-col block per image:
                    #   block j: cols 9j+0:4 coord hi, 9j+4:8 coord lo,
                    #   col 9j+8 matched indicator (1.0 on that image's half)
                    GW = qpool.tile([P, 8], F32, name="GW", tag="gw", bufs=3)
                    GWB = qpool.tile([P, 18], BF16, name="GWB", tag="gwb", bufs=3)
                    nc.vector.memset(GW[:, :], 0.0)
                    for q, T_ in enumerate((TX1, TY1, TX2, TY2)):
                        nc.vector.tensor_copy(GW[0:64, q : q + 1], T_[0:64, :])
                        nc.vector.tensor_copy(GW[64:P, 4 + q : 5 + q], T_[64:P, :])
                    gwv = GW.rearrange("p (h q) -> p h q", h=2)      # [P, 2, 4]
                    gbv = GWB.rearrange("p (h x) -> p h x", h=2)     # [P, 2, 9]
                    nc.vector.tensor_copy(gbv[:, 0, 0:4], gwv[:, 0])   # A hi
                    nc.vector.tensor_copy(gbv[:, 1, 0:4], gwv[:, 1])   # B hi
                    nc.vector.tensor_tensor(gbv[:, 0, 4:8], gwv[:, 0], gbv[:, 0, 0:4],
                                            ALU.subtract)            # A lo
                    nc.vector.tensor_tensor(gbv[:, 1, 4:8], gwv[:, 1], gbv[:, 1, 0:4],
                                            ALU.subtract)            # B lo
                    nc.vector.tensor_copy(gbv[:, 0, 8:9], OAB[:, 0:1])
                    nc.vector.tensor_copy(gbv[:, 1, 8:9], OAB[:, 1:2])

                    # ================= pairwise chunk loop =================
                    for n0, nc_ in CHUNKS:
                        nj = nc_ // 32
                        PX2 = pspool.tile([P, NC], F32, name="PX2", tag="st", bufs=3)
                        PX1 = pspool.tile([P, NC], F32, name="PX1", tag="st", bufs=3)
                        PY2 = pspool.tile([P, NC], F32, name="PY2", tag="st", bufs=3)
                        PY1 = pspool.tile([P, NC], F32, name="PY1", tag="st", bufs=3)
                        PA1 = pspool.tile([P, NC], F32, name="PA1", tag="st", bufs=3)
                        for s, PT_ in enumerate((PX2, PX1, PY2, PY1, PA1)):
                            for j0 in range(0, nc_, 512):
                                jn = min(512, nc_ - j0)
                                rhs = STGB[0:20, n0 + j0 : n0 + j0 + jn]
                                nc.tensor.matmul(
                                    PT_[:, j0 : j0 + jn],
                                    SELS[:, s * P : (s + 1) * P],
                                    rhs, start=True, stop=True,
                                )
                        # ScalarE: relu evac with per-partition bias
                        AXB = wpool.tile([P, 2 * NC], F32, name="AXB", tag="axb")
                        AYB = wpool.tile([P, 2 * NC], F32, name="AYB", tag="ayb")
                        S3 = wpool.tile([P, NC], F32, name="S3", tag="s3")
                        nc.scalar.activation(AXB[:, 0:nc_], PX2[:, 0:nc_],
                                             ACT.Relu, bias=NX1[:, :])
                        nc.scalar.activation(AXB[:, NC : NC + nc_], PX1[:, 0:nc_],
                                             ACT.Relu, bias=NX1[:, :])
                        nc.scalar.activation(AYB[:, 0:nc_], PY2[:, 0:nc_],
                                             ACT.Relu, bias=NY1[:, :])
                        nc.scalar.activation(AYB[:, NC : NC + nc_], PY1[:, 0:nc_],
                                             ACT.Relu, bias=NY1[:, :])
                        nc.scalar.activation(S3[:, 0:nc_], PA1[:, 0:nc_],
                                             ACT.Identity, bias=A2S[:, :])
                        # overlaps / inter / ratio
                        CX = wpool.tile([P, NC], F32, name="CX", tag="cx")
                        CY = wpool.tile([P, NC], F32, name="CY", tag="cy")
                        CYR = wpool.tile([P, NC], F32, name="CYR", tag="cyr", bufs=1)
                        NUM = wpool.tile([P, NC], F32, name="NUM", tag="num", bufs=1)
                        RH = wpool.tile([P, NC], F32, name="RH", tag="rh", bufs=1)
                        nc.vector.scalar_tensor_tensor(
                            CX[:, 0:nc_], AXB[:, 0:nc_], wg, AXB[:, NC : NC + nc_],
                            ALU.min, ALU.subtract,
                        )
                        nc.vector.scalar_tensor_tensor(
                            CY[:, 0:nc_], AYB[:, 0:nc_], hg, AYB[:, NC : NC + nc_],
                            ALU.min, ALU.subtract,
                        )
                        nc.scalar.activation(CYR[:, 0:nc_], CY[:, 0:nc_], ACT.Relu)
                        nc.vector.scalar_tensor_tensor(
                            NUM[:, 0:nc_], CX[:, 0:nc_], 0.0, CYR[:, 0:nc_],
                            ALU.max, ALU.mult,
                        )
                        Q = wpool.tile([P, NC], F32, name="Q", tag="q", bufs=1)
                        nc.vector.reciprocal_approx_fast(Q[:, 0:nc_], S3[:, 0:nc_])
                        nc.vector.tensor_tensor(RH[:, 0:nc_], NUM[:, 0:nc_],
                                                Q[:, 0:nc_], ALU.mult)
                        # transpose-space argmax over targets
                        T = wpool.tile([P, NC], F32, name="T", tag="t", bufs=1)
                        TRD = wpool.tile([P, NC // 32], F32, name="TRD", tag="trd")
                        B4 = wpool.tile([P, NC // 32], F32, name="B4", tag="b4")
                        nc.vector.transpose(T[:, 0:nc_], RH[:, 0:nc_])
                        nc.vector.tensor_reduce(
                            TRD[:, 0:nj],
                            T[:, 0:nc_].rearrange("p (j r) -> p j r", r=32),
                            mybir.AxisListType.X, ALU.max,
                        )
                        # fold target halves: shift blocks {32:64,96:128} down
                        # via DMA so the pairwise max is base-aligned, then
                        # duplicate the folded rows back up.
                        TRB = wpool.tile([P, NC // 32], F32, name="TRB", tag="trb",
                                         bufs=1)
                        nc.sync.dma_start(out=TRB[0:32, 0:nj], in_=TRD[32:64, 0:nj])
                        nc.sync.dma_start(out=TRB[64:96, 0:nj], in_=TRD[96:P, 0:nj])
                        nc.vector.tensor_tensor(B4[0:32, 0:nj], TRD[0:32, 0:nj],
                                                TRB[0:32, 0:nj], ALU.max)
                        nc.vector.tensor_tensor(B4[64:96, 0:nj], TRD[64:96, 0:nj],
                                                TRB[64:96, 0:nj], ALU.max)
                        nc.sync.dma_start(out=B4[32:64, 0:nj], in_=B4[0:32, 0:nj])
                        nc.sync.dma_start(out=B4[96:P, 0:nj], in_=B4[64:96, 0:nj])
                        MASKT = wpool.tile([P, NC], BF16, name="MASKT", tag="mt", bufs=1)
                        MASKM = wpool.tile([P, NC], BF16, name="MASKM", tag="mm", bufs=1)
                        MASK = wpool.tile([P, NC], BF16, name="MASK", tag="msk")
                        nc.vector.tensor_tensor(
                            MASKT[:, 0:nc_].rearrange("p (j r) -> p j r", r=32),
                            T[:, 0:nc_].rearrange("p (j r) -> p j r", r=32),
                            B4[:, 0:nj, None].broadcast_to([P, nj, 32]),
                            ALU.is_equal,
                        )
                        # zero mask where not matched; flags ride the gather
                        nc.vector.scalar_tensor_tensor(
                            MASKM[:, 0:nc_], T[:, 0:nc_], 1.0, MASKT[:, 0:nc_],
                            ALU.is_gt, ALU.mult,
                        )
                        nc.vector.transpose(MASK[:, 0:nc_], MASKM[:, 0:nc_])
                        # PE gather: [18, nc_] = GWB^T @ mask
                        GC = pspool.tile([18, NC], F32, name="GC", tag="gc", bufs=1)
                        for j0 in range(0, nc_, 512):
                            jn = min(512, nc_ - j0)
                            nc.tensor.matmul(GC[:, j0 : j0 + jn], GWB[:, :],
                                             MASK[:, j0 : j0 + jn],
                                             start=True, stop=True)
                        GCB = wpool.tile([18, NC], BF16, name="GCB", tag="gcb")
                        nc.scalar.activation(GCB[:, 0:nc_], GC[:, 0:nc_], ACT.Copy)
                        nc.sync.dma_start(out=STGF[0:18, n0 : n0 + nc_],
                                          in_=GCB[:, 0:nc_])

                    # ============== return to n-part layout ==============
                    m9v = MT9.rearrange("p (b q c) -> p b q c", b=bc, q=9)
                    for j, bslot in enumerate((bA, bB)):
                        for q in range(9):
                            src = STGF[9 * j + q : 9 * j + q + 1, :].rearrange(
                                "o (p c) -> o p c", c=C)
                            nc.sync.dma_start(out=m9v[:, bslot, q], in_=src)

            with (
                tc.tile_pool(name="fin", bufs=1) as wpool,
                tc.tile_pool(name="fpsum", bufs=1, space="PSUM") as pspool,
            ):
                # ================= batched finalization =================
                BCC = bc * C
                pv = PRED.rearrange("p (b c f) -> p b c f", b=bc, f=5)
                L = pv[:, :, :, 4]      # logits [128, bc, 66]
                CXp = pv[:, :, :, 0]
                CYp = pv[:, :, :, 1]
                Wp = pv[:, :, :, 2]
                Hp = pv[:, :, :, 3]
                X2 = crd[:, :, 0]       # [P, bc, C] views
                X1 = crd[:, :, 1]
                Y2 = crd[:, :, 2]
                Y1 = crd[:, :, 3]
                A1S = crd[:, :, 4]
                m9 = MT9.rearrange("p (b q c) -> p b q c", b=bc, q=9)

                def ftile(name, tag=None, bufs=None):
                    return wpool.tile([P, BCC], F32, name=name, tag=tag or name,
                                      bufs=bufs or 1)

                # matched target coords = hi + lo, [P, bc, 4, C] -> [P, bc*4*C]
                MT = ppool.tile([P, bc * 4 * C], F32, name="MT")
                mtv = MT.rearrange("p (b q c) -> p b q c", b=bc, q=4)
                nc.vector.tensor_tensor(mtv[:, :, :, :], m9[:, :, 0:4, :],
                                        m9[:, :, 4:8, :], ALU.add)
                GX1 = mtv[:, :, 0]
                GY1 = mtv[:, :, 1]
                GX2 = mtv[:, :, 2]
                GY2 = mtv[:, :, 3]

                MTC = ppool.tile([P, BCC], F32, name="MTC")   # matched 0/1
                nc.vector.tensor_scalar(
                    MTC.rearrange("p (b c) -> p b c", b=bc), m9[:, :, 8, :], 0.5,
                    None, ALU.is_gt)
                bview = lambda t: t.rearrange("p (b c) -> p b c", b=bc)

                # ---- focal ----
                AZ = ftile("AZ"); SP = ftile("SP"); U0 = ftile("U0"); ZT = ftile("ZT")
                BCE = ftile("BCE"); PT = ftile("PT"); SQ = ftile("SQ"); FF = ftile("FF")
                nc.scalar.activation(AZ[:, :], L, ACT.Abs)
                # softplus(-|z|) = ln(1 + exp(-|z|))  (Softplus not in CoreSim)
                nc.scalar.activation(SP[:, :], AZ[:, :], ACT.Exp, scale=-1.0)
                nc.scalar.activation(SP[:, :], SP[:, :], ACT.Ln, bias=1.0)
                nc.vector.scalar_tensor_tensor(U0[:, :], L, 0.0, SP[:, :], ALU.max, ALU.add)
                nc.vector.tensor_tensor(ZT[:, :], L, MTC[:, :], ALU.mult)
                nc.vector.tensor_tensor(BCE[:, :], U0[:, :], ZT[:, :], ALU.subtract)
                nc.scalar.activation(PT[:, :], BCE[:, :], ACT.Exp, scale=-1.0)
                nc.scalar.activation(SQ[:, :], PT[:, :], ACT.Square, bias=NEG1[:, :])
                nc.vector.scalar_tensor_tensor(FF[:, :], SQ[:, :], 0.25, BCE[:, :],
                                               ALU.mult, ALU.mult)
                nc.vector.tensor_reduce(SC[:, 0:bc], bview(FF), mybir.AxisListType.X,
                                        ALU.add)

                # ---- CIoU ----
                T1 = ftile("T1"); T2 = ftile("T2"); T3 = ftile("T3"); T4 = ftile("T4")
                IW = ftile("IW"); IH = ftile("IH"); IN2 = ftile("IN2"); AG = ftile("AG")
                UN = ftile("UN"); QU = ftile("QU"); IOU = ftile("IOU")
                DX = ftile("DX"); DY = ftile("DY"); DG = ftile("DG"); QD = ftile("QD")
                DD = ftile("DD"); DIOU = ftile("DIOU")
                WGE = ftile("WGE"); HGE = ftile("HGE"); QH = ftile("QH"); RG = ftile("RG")
                ATG = ftile("ATG"); ATP = ftile("ATP"); VV = ftile("VV"); DEN = ftile("DEN")
                QA = ftile("QA"); AL = ftile("AL"); AV = ftile("AV"); CIO = ftile("CIO")
                MC = ftile("MC"); A1R = ftile("A1R")

                # intersection with matched boxes
                nc.vector.tensor_tensor(bview(T1), X1, GX1, ALU.max)
                nc.vector.tensor_tensor(bview(T2), X2, GX2, ALU.min)
                nc.vector.tensor_tensor(IW[:, :], T2[:, :], T1[:, :], ALU.subtract)
                nc.vector.tensor_tensor(bview(T3), Y1, GY1, ALU.max)
                nc.vector.tensor_tensor(bview(T4), Y2, GY2, ALU.min)
                nc.vector.tensor_tensor(IH[:, :], T4[:, :], T3[:, :], ALU.subtract)
                nc.vector.tensor_scalar(IH[:, :], IH[:, :], 0.0, None, ALU.max)
                nc.vector.scalar_tensor_tensor(IN2[:, :], IW[:, :], 0.0, IH[:, :],
                                               ALU.max, ALU.mult)
                # union = a1 + ag - inter   (A1S = (3/13) a1)
                nc.vector.tensor_tensor(bview(WGE), GX2, GX1, ALU.subtract)
                nc.vector.tensor_tensor(bview(HGE), GY2, GY1, ALU.subtract)
                nc.vector.tensor_tensor(AG[:, :], WGE[:, :], HGE[:, :], ALU.mult)
                nc.vector.tensor_scalar(bview(A1R), A1S, 13.0 / 3.0, None, ALU.mult)
                nc.vector.tensor_tensor(UN[:, :], A1R[:, :], AG[:, :], ALU.add)
                nc.vector.scalar_tensor_tensor(UN[:, :], UN[:, :], EPS, IN2[:, :],
                                               ALU.add, ALU.subtract)
                nc.vector.reciprocal_approx_fast(QU[:, :], UN[:, :])
                nc.vector.tensor_tensor(IOU[:, :], IN2[:, :], QU[:, :], ALU.mult)
                # enclosing diag
                nc.vector.tensor_tensor(bview(T1), X1, GX1, ALU.min)
                nc.vector.tensor_tensor(bview(T2), X2, GX2, ALU.max)
                nc.vector.tensor_tensor(DX[:, :], T2[:, :], T1[:, :], ALU.subtract)
                nc.vector.tensor_tensor(bview(T3), Y1, GY1, ALU.min)
                nc.vector.tensor_tensor(bview(T4), Y2, GY2, ALU.max)
                nc.vector.tensor_tensor(DY[:, :], T4[:, :], T3[:, :], ALU.subtract)
                nc.scalar.activation(T1[:, :], DX[:, :], ACT.Square)
                nc.scalar.activation(T2[:, :], DY[:, :], ACT.Square)
                nc.vector.scalar_tensor_tensor(DG[:, :], T1[:, :], EPS, T2[:, :],
                                               ALU.add, ALU.add)
                nc.vector.reciprocal_approx_fast(QD[:, :], DG[:, :])
                # center distance
                nc.vector.tensor_tensor(bview(T3), GX1, GX2, ALU.add)
                nc.vector.scalar_tensor_tensor(bview(T3), bview(T3)[:, :], 0.5, CXp,
                                               ALU.mult, ALU.subtract)
                nc.vector.tensor_tensor(bview(T4), GY1, GY2, ALU.add)
                nc.vector.scalar_tensor_tensor(bview(T4), bview(T4)[:, :], 0.5, CYp,
                                               ALU.mult, ALU.subtract)
                nc.scalar.activation(T3[:, :], T3[:, :], ACT.Square)
                nc.scalar.activation(T4[:, :], T4[:, :], ACT.Square)
                nc.vector.tensor_tensor(DD[:, :], T3[:, :], T4[:, :], ALU.add)
                nc.vector.tensor_tensor(DD[:, :], DD[:, :], QD[:, :], ALU.mult)
                # diou - 1 = dist/diag - iou
                nc.vector.scalar_tensor_tensor(DIOU[:, :], IOU[:, :], -1.0, DD[:, :],
                                               ALU.mult, ALU.add)
                # aspect term.  ScalarE Arctan domain is [-pi/2, pi/2], so use
                # arctan(x) = a + 1[x>1]*(pi/2 - 2a),  a = arctan(min(x, 1/x)).
                def atan_pos(dst, x, ta, tb):
                    nc.vector.tensor_scalar(ta[:, :], x[:, :], 1e-20, None, ALU.max)
                    nc.vector.reciprocal_approx_fast(tb[:, :], ta[:, :])
                    nc.vector.tensor_tensor(tb[:, :], ta[:, :], tb[:, :], ALU.min)
                    nc.scalar.activation(dst[:, :], tb[:, :], ACT.Arctan)
                    nc.vector.tensor_scalar(ta[:, :], ta[:, :], 1.0, None, ALU.is_gt)
                    nc.vector.tensor_scalar(tb[:, :], dst[:, :], -2.0, float(np.pi / 2),
                                            ALU.mult, ALU.add)
                    nc.vector.tensor_tensor(ta[:, :], ta[:, :], tb[:, :], ALU.mult)
                    nc.vector.tensor_tensor(dst[:, :], dst[:, :], ta[:, :], ALU.add)

                nc.vector.tensor_scalar(T1[:, :], HGE[:, :], 1e-12, None, ALU.max)
                nc.vector.reciprocal_approx_fast(QH[:, :], T1[:, :])
                nc.vector.tensor_tensor(RG[:, :], WGE[:, :], QH[:, :], ALU.mult)
                atan_pos(ATG, RG, T1, T2)
                nc.vector.tensor_scalar(bview(T2), Hp, 1e-12, None, ALU.max)
                nc.vector.reciprocal_approx_fast(QH[:, :], T2[:, :])
                nc.vector.scalar_tensor_tensor(bview(RG), bview(QH)[:, :], 1.0, Wp,
                                               ALU.mult, ALU.mult)
                atan_pos(ATP, RG, T1, T2)
                nc.vector.tensor_tensor(T3[:, :], ATG[:, :], ATP[:, :], ALU.subtract)
                nc.scalar.activation(VV[:, :], T3[:, :], ACT.Square,
                                     scale=2.0 / np.pi)
                # alpha = v / (1 - iou + v + eps)
                nc.vector.tensor_tensor(DEN[:, :], VV[:, :], IOU[:, :], ALU.subtract)
                nc.vector.tensor_scalar(DEN[:, :], DEN[:, :], 1.0 + EPS, None, ALU.add)
                nc.vector.reciprocal_approx_fast(QA[:, :], DEN[:, :])
                nc.vector.tensor_tensor(AL[:, :], VV[:, :], QA[:, :], ALU.mult)
                nc.vector.tensor_tensor(AV[:, :], AL[:, :], VV[:, :], ALU.mult)
                # ciou = 1 + (diou - 1) + alpha*v
                nc.vector.scalar_tensor_tensor(CIO[:, :], DIOU[:, :], 1.0, AV[:, :],
                                               ALU.add, ALU.add)
                nc.vector.tensor_tensor(MC[:, :], CIO[:, :], MTC[:, :], ALU.mult)
                nc.vector.tensor_reduce(SC[:, bc : 2 * bc], bview(MC),
                                        mybir.AxisListType.X, ALU.add)
                nc.vector.tensor_reduce(SC[:, 2 * bc : 3 * bc], bview(MTC),
                                        mybir.AxisListType.X, ALU.add)

                # ---- cross-partition reduce + output ----
                PS = pspool.tile([1, 3 * bc], F32, name="PS", tag="ps")
                nc.tensor.matmul(PS[:, :], ONES[:, :], SC[:, :], start=True, stop=True)
                OUTS = wpool.tile([1, 3 * bc], F32, name="OUTS", tag="outs")
                nc.scalar.activation(OUTS[:, :], PS[:, :], ACT.Copy)
                nc.sync.dma_start(out=out_d[:, :], in_=OUTS[:, :])

    nc.finalize()
    return nc


# ---------------- host side ----------------
_CACHE = {}


def _get_nc():
    if "nc" not in _CACHE:
        _CACHE["nc"] = build_nc()
    return _CACHE["nc"]


def combine(per_img):
    """per_img [B, 3] float64: (focal_sum, masked_ciou_sum, n_pos) -> loss."""
    f = per_img[:, 0] / float(N)
    conf = f.mean()
    npos = per_img[:, 2]
    per_box = per_img[:, 1] / np.maximum(npos, 1.0)
    has = (npos > 0).astype(np.float64)
    nimg = has.sum()
    box = (per_box * has).sum() / max(nimg, 1.0)
    return np.float32(conf + 7.5 * box)


def run(preds, targets, **spmd_kwargs):
    from concourse.bass_utils import run_bass_kernel_spmd

    preds = np.ascontiguousarray(preds, np.float32)
    targets = np.ascontiguousarray(targets, np.float32)
    nc = _get_nc()
    sels, onesneg, onesab = host_consts()
    in_maps = [
        {
            "preds": pad_preds(preds[c * BC : (c + 1) * BC]),
            "targets": np.ascontiguousarray(targets[c * BC : (c + 1) * BC]),
            "sels": sels,
            "onesneg": onesneg,
            "onesab": onesab,
        }
        for c in range(NCORES)
    ]
    res = run_bass_kernel_spmd(nc, in_maps, list(range(NCORES)), **spmd_kwargs)
    rows = []
    for c in range(NCORES):
        o = np.asarray(res.results[c]["out"], np.float64).reshape(3, BC)
        rows.append(o.T)  # [BC, 3]
    per_img = np.concatenate(rows, 0)
    return per_img, res


def kernel(preds, targets):
    per_img, _ = run(preds, targets)
    return combine(per_img)
